# revision 38
# baseline (speedup 1.0000x reference)
"""Trainium2 Bass kernel for the sparse-attention nn module (nn_BDH_48421461295735).

Strategy: 8 NeuronCores = 8 (batch, head) pairs (B=2 x NH=4).  Each core runs
all 4 layers for its head; the only cross-core traffic is a per-layer
AllReduce (within each batch's group of 4 cores) of the per-head decoder
partial p = (x_sparse*y_sparse) @ dec_h, chunked by 512 token rows (bf16).

v4 over v2 (1.15 ms):
  - optional fp8e4+DoubleRow for the p1 (wenc @ xT), y_sparse
    (wencv @ ykvT) and ykv (x^T @ scoresT) matmuls, halving their PE
    cycles.  Flags P1_FP8 / YS_FP8 / YKV_FP8 pick the set (the lm_head
    and final-x stay bf16: quantizing them costs ~1.8e-2 rel err).
  - p1's fp8 xT is produced by converting the bf16 DMA-transpose output;
    the converts are DEFERRED one pipeline iteration (emitted at the top
    of the next p13) so p4's ACT ops never head-of-line block p2's
    score-staging copies (the v2/v3 traces showed ~10us PE stalls per
    chunk from exactly that).
  - p2 emits the ykvTp copies before the stats squares so p3's first
    y_sparse matmul is not queued behind them.
  - final lm_head chunks reordered so fin(ts) never waits on the last
    p4's transposes.
"""

import math
import sys

import numpy as np

for _p in ("/opt/trn_rl_repo",):
    if _p not in sys.path:
        sys.path.insert(0, _p)

import concourse.bass as bass
import concourse.bacc as bacc
import concourse.mybir as mybir
import concourse.tile as tile
from concourse.tile_rust import add_dep_helper
from concourse.bass_utils import run_bass_kernel_spmd

F32 = mybir.dt.float32
BF16 = mybir.dt.bfloat16
FP8 = mybir.dt.float8e4
AF = mybir.ActivationFunctionType
ALU = mybir.AluOpType
DR = mybir.MatmulPerfMode.DoubleRow

FULL_CFG = dict(T=2048, D=256, N=2048, NL=4, V=256, NH=4, B=2)
P = 128
SUP = 512
EPS = 1e-5
YKV_SC = 64.0  # ykvT pre-scale so fp8e4 never saturates (folded into rs4)

P1_FP8 = False   # x_sparse matmul via fp8 DoubleRow (wenc, xT in fp8)
YS_FP8 = False   # y_sparse matmul via fp8 DoubleRow (wencv, ykvT in fp8)
YKV_FP8 = True   # ykv matmul via fp8 DoubleRow (x, scoresT in fp8)


def build_nc(cfg, mm_dt=BF16, n_cores=8):
    T, D, N, NL, V = cfg["T"], cfg["D"], cfg["N"], cfg["NL"], cfg["V"]
    NH = cfg["NH"]
    assert T % SUP == 0 and D % P == 0 and N % 256 == 0 and V == D
    nTB, nTS, nD, nK = T // P, T // SUP, D // P, N // P
    nJ = nK // 2
    nQ = SUP // P  # 4
    assert nD == 2
    enc_dt = FP8 if P1_FP8 else mm_dt
    ykv_dt = FP8 if YS_FP8 else mm_dt
    st_dt = FP8 if YKV_FP8 else mm_dt

    nc = bacc.Bacc("TRN2", target_bir_lowering=False, debug=False,
                   num_devices=n_cores)

    x0_d = nc.dram_tensor("x0", [T, D], F32, kind="ExternalInput")
    x0b_d = nc.dram_tensor("x0b", [T, D], mm_dt, kind="ExternalInput")
    x0b8_d = nc.dram_tensor("x0b8", [T, D], FP8, kind="ExternalInput")
    x0t_d = nc.dram_tensor("x0t", [D, T], mm_dt, kind="ExternalInput")
    x0tp_d = nc.dram_tensor("x0tp", [P, 2 * T], FP8, kind="ExternalInput")
    wencp_d = nc.dram_tensor("wencp", [P, 2 * N], enc_dt, kind="ExternalInput")
    wencv_d = nc.dram_tensor("wencv", [P, 2 * N], ykv_dt, kind="ExternalInput")
    wdec_d = nc.dram_tensor("wdec", [N, D], mm_dt, kind="ExternalInput")
    wlm_d = nc.dram_tensor("wlm", [D, V], mm_dt, kind="ExternalInput")
    ctab_d = nc.dram_tensor("ctab", [N // 2, T], mm_dt, kind="ExternalInput")
    stab_d = nc.dram_tensor("stab", [N // 2, T], mm_dt, kind="ExternalInput")
    maskt_d = nc.dram_tensor("maskt", [P, P], mm_dt, kind="ExternalInput")
    ident_d = nc.dram_tensor("ident", [P, P], mm_dt, kind="ExternalInput")
    out_d = nc.dram_tensor("out", [T, V], F32, kind="ExternalOutput")

    # AllReduce groups: one group of NH cores per batch.
    RG = [list(range(g * NH, (g + 1) * NH)) for g in range(max(1, n_cores // NH))]

    with tile.TileContext(nc) as tc:
        _keep = []  # keep tc.tile free-closures alive (GC would release pools)

        def ptile(shape, dtype, name, **kw):
            t, free = tc.tile(shape, dtype, name=name, **kw)
            _keep.append(free)
            return t

        # ---- persistent SBUF tensors ----
        wencp_sb = ptile([P, 2, N], enc_dt, name="wencp")  # d-pair layout
        wencvp = ptile([P, 2, N], ykv_dt, name="wencvp")   # d-pair layout
        wdec_sb = [ptile([P, D], mm_dt, name=f"wdec{k}") for k in range(nK)]
        wlm_sb = [ptile([P, V], mm_dt, name=f"wlm{d}") for d in range(nD)]
        maskt_sb = ptile([P, P], mm_dt, name="maskt")
        x_f32 = [ptile([P, D], F32, name=f"xf{t}") for t in range(nTB)]
        # x_bf / xT_bf double-buffered by layer parity (p4 writes the other)
        if not YKV_FP8:
            x_bf = [[ptile([P, D], mm_dt, name=f"xb{pp}_{t}")
                     for t in range(nTB)] for pp in range(2)]
        xT_bf = [[ptile([P, T], mm_dt, name=f"xT{pp}_{d}") for d in range(nD)]
                 for pp in range(2)]
        if P1_FP8:
            # fp8 d-pair transposed x for the p1 DoubleRow matmuls
            xTp = [ptile([P, 2, T], FP8, name=f"xTp{pp}") for pp in range(2)]
        if YKV_FP8:
            # fp8 token-block-pair x for the ykv DoubleRow matmuls
            xb2 = [[ptile([P, 2, D], FP8, name=f"xb2_{pp}_{m}")
                    for m in range(nTB // 2)] for pp in range(2)]
        # qrT pair tiles: [:,0,:] = even half (pair group j), [:,1,:] = odd
        qp = [ptile([P, 2, T], FP8, name=f"qp{j}") for j in range(nJ)]
        # ykvT in d-pairs (scaled by 1/YKV_SC; the scale commutes out
        # through relu/decoder and is folded into rs4)
        ykvTp = ptile([P, 2, T], ykv_dt, name="ykvTp")
        # x_sparse stash, double-buffered by chunk parity
        xs_sb = [[ptile([P, SUP], mm_dt, name=f"xs{g}_{k}") for k in range(nK)]
                 for g in range(2)]
        eps_sb = ptile([P, 1], F32, name="epsb")
        nc.vector.memset(eps_sb[:], EPS)
        epsk_sb = ptile([P, 1], F32, name="epskb")
        nc.vector.memset(epsk_sb[:], EPS / (YKV_SC * YKV_SC))
        ones_sb = ptile([P, 1], mm_dt, name="onesb")
        nc.vector.memset(ones_sb[:], 1.0)
        ident_sb = ptile([P, P], mm_dt, name="identb")

        # per-layer DRAM bounce buffers for the chunked AllReduce (bf16)
        p_loc = [ptile([T, D], mm_dt, space="DRAM", name=f"ploc{l}")
                 for l in range(NL)]
        p_sum = [ptile([T, D], mm_dt, space="DRAM", addr_space="Shared",
                       name=f"psum{l}") for l in range(NL)]

        # transient pools
        _cms = [tc.tile_pool(name="spT", bufs=4),     # rope tables
                tc.tile_pool(name="spR", bufs=3),     # rope temporaries
                tc.tile_pool(name="spS", bufs=4),     # st staging, xy
                tc.tile_pool(name="spC", bufs=2),     # pch/pin/och staging
                tc.tile_pool(name="spL", bufs=8),     # LN scalars + scratch
                tc.tile_pool(name="ppb", bufs=4, space="PSUM"),
                tc.tile_pool(name="ppw", bufs=2, space="PSUM"),
                tc.tile_pool(name="pss", bufs=1, space="PSUM"),
                tc.tile_pool(name="ppr", bufs=1, space="PSUM")]
        spT, spR, spS, spC, spL, ppb, ppw, pss, ppr = \
            [cm.__enter__() for cm in _cms]

        def ln_stats_nm(src_ap):
            """LN stats over free dim D of [P, D] f32/psum AP.
            Returns (nm, rs): per-partition -mu and 1/sd."""
            s1 = spL.tile([P, 1], F32, tag="ln1", name="s1")
            nc.vector.reduce_sum(s1[:], src_ap, axis=mybir.AxisListType.X)
            nm = spL.tile([P, 1], F32, tag="ln2", name="nm")
            nc.vector.tensor_scalar_mul(nm[:], s1[:], -1.0 / D)
            sq = spL.tile([P, D], BF16, tag="lnsq", name="sq", bufs=2)
            ss = spL.tile([P, 1], F32, tag="ln3", name="ss")
            nc.scalar.activation(sq[:], src_ap, AF.Square, bias=nm[:],
                                 accum_out=ss[:])
            sd = spL.tile([P, 1], F32, tag="ln4", name="sd")
            nc.scalar.activation(sd[:], ss[:], AF.Sqrt, bias=eps_sb[:],
                                 scale=1.0 / D)
            rs = spL.tile([P, 1], F32, tag="ln5", name="rs")
            nc.vector.reciprocal(rs[:], sd[:])
            return nm, rs

        # ---- setup: pure DMAs; only what p1(0,0)/p2(0,0) needs goes on the
        # sync queue (which also carries the first rope tables) ----
        with nc.named_scope("setup"):
            nc.sync.dma_start(
                wencp_sb[:].rearrange("p a n -> p (a n)"), wencp_d[:, :])
            if P1_FP8:
                nc.sync.dma_start(
                    xTp[0][:].rearrange("p a t -> p (a t)"), x0tp_d[:, :])
            else:
                for d in range(nD):
                    nc.sync.dma_start(xT_bf[0][d][:],
                                      x0t_d[d * P:(d + 1) * P, :])
            if YKV_FP8:
                for m in range(nTB // 2):
                    for h in range(2):
                        t = 2 * m + h
                        nc.scalar.dma_start(xb2[0][m][:, h, :],
                                            x0b8_d[t * P:(t + 1) * P, :])
            else:
                for t in range(nTB):
                    nc.scalar.dma_start(x_bf[0][t][:],
                                        x0b_d[t * P:(t + 1) * P, :])
            nc.scalar.dma_start(maskt_sb[:], maskt_d[:, :])

        def setup_late():
            # needed only from p3(0,0) onwards: emitted after p1(0,0) so
            # chunk 0's relus aren't queued behind these DMA issues
            with nc.named_scope("setup2"):
                nc.scalar.dma_start(
                    wencvp[:].rearrange("p a n -> p (a n)"), wencv_d[:, :])
                for k in range(nK):
                    nc.scalar.dma_start(wdec_sb[k][:],
                                        wdec_d[k * P:(k + 1) * P, :])
                for t in range(nTB):
                    nc.scalar.dma_start(x_f32[t][:],
                                        x0_d[t * P:(t + 1) * P, :])
                for d in range(nD):
                    nc.scalar.dma_start(wlm_sb[d][:],
                                        wlm_d[d * P:(d + 1) * P, :])
                nc.scalar.dma_start(ident_sb[:], ident_d[:, :])

        def rope_j(j, l1, xs, c0, c1):
            """x_sparse matmuls for pair-group j + relu + rope (v2 scheme:
            DVE reads at most one SBUF tensor; gpsimd does SBUF*SBUF)."""
            par = l1 % 2
            ct = spT.tile([P, SUP], mm_dt, tag="ctc", name="ct")
            st = spT.tile([P, SUP], mm_dt, tag="stc", name="st")
            nc.scalar.dma_start(ct[:], ctab_d[j * P:(j + 1) * P, c0:c1])
            nc.scalar.dma_start(st[:], stab_d[j * P:(j + 1) * P, c0:c1])
            psA = ppb.tile([P, SUP], F32, tag="big", name="psA")
            psB = ppb.tile([P, SUP], F32, tag="big", name="psB")
            if P1_FP8:
                xT = xTp[par]
                nc.tensor.matmul(psA[:], wencp_sb[:, :, j * P:(j + 1) * P],
                                 xT[:, :, c0:c1], start=True, stop=True,
                                 perf_mode=DR)
                nc.tensor.matmul(
                    psB[:], wencp_sb[:, :, (j + nJ) * P:(j + nJ + 1) * P],
                    xT[:, :, c0:c1], start=True, stop=True, perf_mode=DR)
            else:
                xT = xT_bf[par]
                for d in range(nD):
                    nc.tensor.matmul(
                        psA[:], wencp_sb[:, d, j * P:(j + 1) * P],
                        xT[d][:, c0:c1], start=(d == 0), stop=(d == nD - 1))
                for d in range(nD):
                    nc.tensor.matmul(
                        psB[:], wencp_sb[:, d, (j + nJ) * P:(j + nJ + 1) * P],
                        xT[d][:, c0:c1], start=(d == 0), stop=(d == nD - 1))
            xsE, xsO = xs[j], xs[j + nJ]
            nc.scalar.activation(xsE[:], psA[:], AF.Relu)
            nc.scalar.activation(xsO[:], psB[:], AF.Relu)
            # rope: qE = relu(A)*c - relu(B)*s ; qO = relu(B)*c + relu(A)*s
            t0 = ppr.tile([P, SUP], F32, tag="tr", name="t0")
            t1 = spR.tile([P, SUP], mm_dt, tag="t1", name="t1")
            t2 = spR.tile([P, SUP], mm_dt, tag="t2", name="t2")
            qE, qO = qp[j][:, 0, c0:c1], qp[j][:, 1, c0:c1]
            nc.vector.scalar_tensor_tensor(t0[:], psA[:], 0.0, ct[:],
                                           ALU.max, ALU.mult)
            nc.gpsimd.tensor_tensor(t1[:], xsO[:], st[:], ALU.mult)
            nc.vector.tensor_tensor(qE, t0[:], t1[:], ALU.subtract)
            nc.gpsimd.tensor_tensor(t2[:], xsO[:], ct[:], ALU.mult)
            t3 = ppr.tile([P, SUP], F32, tag="tr", name="t3")
            nc.vector.scalar_tensor_tensor(t3[:], psA[:], 0.0, st[:],
                                           ALU.max, ALU.mult)
            nc.vector.tensor_tensor(qO, t3[:], t2[:], ALU.add)

        def emit_p1(l, ts):
            xs = xs_sb[ts % 2]
            c0, c1 = ts * SUP, (ts + 1) * SUP
            with nc.named_scope(f"l{l}c{ts}_p1"):
                for j in range(nJ):
                    rope_j(j, l, xs, c0, c1)

        _rs4 = [None]

        def emit_p2(l, ts):
            par = l % 2
            c0, c1 = ts * SUP, (ts + 1) * SUP
            with nc.named_scope(f"l{l}c{ts}_p2"):
                # ykv is computed directly in transposed layout:
                #   ykvT[d, t] = sum_s x[s, d] * scoresT[s, t]
                # The per-token LN of ykv needs no mean (ykv is exactly
                # zero-mean since x is layer-normed), and the 1/sd scale
                # commutes through relu/encv/decoder, so it is applied to
                # p's rows (per-partition scale) in p3's pch copy instead.
                yT_ps = [ppw.tile([P, SUP], F32, tag="wide", name=f"yT{d}")
                         for d in range(nD)]
                nsb = nQ * ts + nQ

                def score_mms(sb):
                    r = sb - nQ * ts
                    q0 = max(0, r)
                    st_ps = ppb.tile([P, SUP], F32, tag="big", name="st_ps")
                    dst = st_ps[:, q0 * P:SUP]
                    for j in range(nJ):
                        nc.tensor.matmul(
                            dst, qp[j][:, :, sb * P:(sb + 1) * P],
                            qp[j][:, :, c0 + q0 * P:c1],
                            start=(j == 0), stop=(j == nJ - 1),
                            perf_mode=DR)
                    return st_ps, r

                def stage_copy(dsl, st_ps, r):
                    # masked copy of one score block into SBUF staging;
                    # dsl(a, b) yields the [P, b-a] destination column slice
                    if r >= 0:
                        if r > 0:
                            nc.gpsimd.memset(dsl(0, r * P), 0.0)
                        nc.vector.tensor_tensor(
                            dsl(r * P, (r + 1) * P),
                            st_ps[:, r * P:(r + 1) * P], maskt_sb[:],
                            ALU.mult)
                        if r + 1 < nQ:
                            nc.scalar.activation(dsl((r + 1) * P, SUP),
                                                 st_ps[:, (r + 1) * P:SUP],
                                                 AF.Copy)
                    else:
                        nc.scalar.activation(dsl(0, SUP), st_ps[:], AF.Copy)

                if YKV_FP8:
                    xb = xb2[par]
                    prev = None
                    for pbi in range(nsb // 2):
                        stp = spS.tile([P, 2, SUP], FP8, tag="stp",
                                       name="stp")
                        for h in range(2):
                            st_ps, r = score_mms(2 * pbi + h)
                            stage_copy(lambda a, b, hh=h: stp[:, hh, a:b],
                                       st_ps, r)
                        # ykv lags one score pair so the PE never waits on
                        # the staging copies it just requested
                        if prev is not None:
                            ppi, pstp = prev
                            for d in range(nD):
                                nc.tensor.matmul(
                                    yT_ps[d][:],
                                    xb[ppi][:, :, d * P:(d + 1) * P],
                                    pstp[:, :, :],
                                    start=(ppi == 0), stop=False,
                                    perf_mode=DR)
                        prev = (pbi, stp)
                    ppi, pstp = prev
                    for d in range(nD):
                        nc.tensor.matmul(
                            yT_ps[d][:], xb[ppi][:, :, d * P:(d + 1) * P],
                            pstp[:, :, :],
                            start=(ppi == 0), stop=True, perf_mode=DR)
                else:
                    xb = x_bf[par]
                    prev = None
                    for si in range(nsb):
                        st_ps, r = score_mms(si)
                        st_sb = spS.tile([P, SUP], mm_dt, tag="stp",
                                         name="st_sb")
                        stage_copy(lambda a, b: st_sb[:, a:b], st_ps, r)
                        if prev is not None:
                            psi, pst = prev
                            for d in range(nD):
                                nc.tensor.matmul(
                                    yT_ps[d][:],
                                    xb[psi][:, d * P:(d + 1) * P],
                                    pst[:], start=(psi == 0), stop=False)
                        prev = (si, st_sb)
                    psi, pst = prev
                    for d in range(nD):
                        nc.tensor.matmul(
                            yT_ps[d][:], xb[psi][:, d * P:(d + 1) * P],
                            pst[:], start=(psi == 0), stop=True)

                # ykvTp copies first: p3's first y_sparse matmul needs them
                for d in range(nD):
                    nc.scalar.activation(ykvTp[:, d, c0:c1], yT_ps[d][:],
                                         AF.Copy, scale=1.0 / YKV_SC)
                # stats: ss4[:, q] = sum_d ykvT^2 via ones-matmul, then
                # rs4 = 1/sqrt(ss/D + eps) stays in token-column layout.
                sqs = []
                for d in range(nD):
                    sq = spS.tile([P, SUP], mm_dt, tag=f"sq{d}", name="sq",
                                  bufs=2)
                    nc.scalar.activation(sq[:], yT_ps[d][:], AF.Square)
                    sqs.append(sq)
                ss4 = pss.tile([P, nQ], F32, tag="ss", name="ss4")
                for q in range(nQ):
                    for d in range(nD):
                        nc.tensor.matmul(
                            ss4[:, q:q + 1], sqs[d][:, q * P:(q + 1) * P],
                            ones_sb[:],
                            start=(q == 0 and d == 0),
                            stop=(q == nQ - 1 and d == nD - 1))
                sd4 = spL.tile([P, nQ], F32, tag="sd4", name="sd4")
                nc.scalar.activation(sd4[:], ss4[:], AF.Sqrt, bias=epsk_sb[:],
                                     scale=1.0 / (D * YKV_SC * YKV_SC))
                rs4 = spL.tile([P, nQ], F32, tag="rs4", name="rs4")
                nc.vector.reciprocal(rs4[:], sd4[:])
                _rs4[0] = rs4

        _last_pch = [None]

        def p3_step(k, xs3, c30, c31, p_ps):
            ys_ps = ppb.tile([P, SUP], F32, tag="big", name="ys_ps")
            if YS_FP8:
                nc.tensor.matmul(ys_ps[:], wencvp[:, :, k * P:(k + 1) * P],
                                 ykvTp[:, :, c30:c31], start=True, stop=True,
                                 perf_mode=DR)
            else:
                for d in range(nD):
                    nc.tensor.matmul(ys_ps[:],
                                     wencvp[:, d, k * P:(k + 1) * P],
                                     ykvTp[:, d, c30:c31],
                                     start=(d == 0), stop=(d == nD - 1))
            xy = spS.tile([P, SUP], mm_dt, tag="xy", name="xy")
            # fused relu+mult on DVE (gpsimd cannot read PSUM)
            nc.vector.scalar_tensor_tensor(
                xy[:], ys_ps[:], 0.0, xs3[k][:], ALU.max, ALU.mult)
            for q in range(nQ):
                nc.tensor.matmul(
                    p_ps[q // 2][:, (q % 2) * D:(q % 2 + 1) * D],
                    xy[:, q * P:(q + 1) * P],
                    wdec_sb[k][:],
                    start=(k == 0 and q % 2 == 0),
                    stop=(k == nK - 1 and q % 2 == 1))

        def p3_tail(p_ps, rs4):
            pch = spC.tile([P, nQ * D], mm_dt, tag="pch", name="pch")
            for q in range(nQ):
                nc.scalar.activation(
                    pch[:, q * D:(q + 1) * D],
                    p_ps[q // 2][:, (q % 2) * D:(q % 2 + 1) * D],
                    AF.Copy, scale=rs4[:, q:q + 1])
            _last_pch[0] = pch

        def emit_p3(l, ts):
            xs3 = xs_sb[ts % 2]
            c30, c31 = ts * SUP, (ts + 1) * SUP
            with nc.named_scope(f"l{l}c{ts}_p3"):
                p_ps = [ppw.tile([P, SUP], F32, tag="wide", name=f"pp{h}")
                        for h in range(nQ // 2)]
                for k in range(nK):
                    p3_step(k, xs3, c30, c31, p_ps)
                p3_tail(p_ps, _rs4[0])

        # deferred bf16->fp8 xT converts: (src_slice, dst_slice) pairs
        _pend_cvt = [[]]

        def flush_cvt():
            for src, dst in _pend_cvt[0]:
                nc.scalar.activation(dst, src, AF.Copy)
            _pend_cvt[0] = []

        def emit_p13(l3, ts3, p1_next):
            """p3 of chunk (l3,ts3) interleaved with p1 of the next chunk."""
            xs3 = xs_sb[ts3 % 2]
            c30, c31 = ts3 * SUP, (ts3 + 1) * SUP
            rs4 = _rs4[0]
            if p1_next is not None:
                l1, ts1 = p1_next
                xs1 = xs_sb[ts1 % 2]
                c10, c11 = ts1 * SUP, (ts1 + 1) * SUP
            with nc.named_scope(f"l{l3}c{ts3}_p31"):
                flush_cvt()
                emit_p4b()
                p_ps = [ppw.tile([P, SUP], F32, tag="wide", name=f"pp{h}")
                        for h in range(nQ // 2)]
                for j in range(nJ):
                    if p1_next is not None:
                        rope_j(j, l1, xs1, c10, c11)
                    p3_step(2 * j, xs3, c30, c31, p_ps)
                    p3_step(2 * j + 1, xs3, c30, c31, p_ps)
                p3_tail(p_ps, rs4)
                flush_tp()

        _pch_dma = [None]

        def emit_ar(l, ts):
            c0, c1 = ts * SUP, (ts + 1) * SUP
            with nc.named_scope(f"l{l}c{ts}_ar"):
                # pch staged by p3 just above (same iteration).  High
                # priority: the collective's issue instruction must be
                # popped by the gpsimd queue the moment pch lands, else
                # the AR start (and its ~14us wire time) slides ~40us to
                # after the next chunk's rope multiplies.
                with tc.high_priority():
                    pd = nc.scalar.dma_start(
                        p_loc[l][c0:c1, :].rearrange("(n p) d -> p n d", p=P),
                        _last_pch[0].rearrange("p (n d) -> p n d", n=nQ))
                    _pch_dma[0] = getattr(pd, "ins", None)
                    nc.gpsimd.collective_compute(
                        "AllReduce", ALU.add, replica_groups=RG,
                        ins=[p_loc[l][c0:c1, :]], outs=[p_sum[l][c0:c1, :]])

        # p4 is split: p4a issues only the pin DMA (one iteration after the
        # AR); p4b runs the LN chain + x writes at the TOP of the p13 two
        # iterations after the AR (when its completion is certain, so the
        # scheduler cannot head-of-line block any queue on it), and the
        # transposes at that p13's BOTTOM (their consumers are one more
        # iteration out).
        _pend_p4 = []   # (l, ts, pin_tile)
        _pend_tp = []   # (dst_slice, src_slice) DMA transposes
        fins = []       # last-layer chunks whose lm_head is pending

        def emit_p4a(l, ts):
            c0, c1 = ts * SUP, (ts + 1) * SUP
            with nc.named_scope(f"l{l}c{ts}_p4a"):
                pin = spC.tile([P, nQ * D], mm_dt, tag="pin", name="pin")
                pi = nc.scalar.dma_start(
                    pin[:].rearrange("p (n d) -> p n d", n=nQ),
                    p_sum[l][c0:c1, :].rearrange("(n p) d -> p n d", p=P))
                # Pin the pin-DMA's *simulated* readiness to ~this
                # iteration's end (the current chunk's pch DMA): the cost
                # model underestimates when the AllReduce lands, and the
                # scheduler would otherwise place the whole AR-gated p4b
                # chain a full iteration early, where it head-of-line
                # blocks the DVE/gpsimd queues for ~10us at every layer
                # boundary.  Runtime cost is nil: its consumers have a
                # full iteration of slack.
                pii = getattr(pi, "ins", None)
                if pii is not None and _pch_dma[0] is not None and nTS > 1:
                    add_dep_helper(pii, _pch_dma[0], sync=False,
                                   reason="pin after current pch (placement)")
                _pend_p4.append((l, ts, pin))

        def emit_p4b(pe_tp=False):
            if not _pend_p4:
                return
            l, ts, pin = _pend_p4.pop(0)
            nxt = (l + 1) % 2
            xbn = None if YKV_FP8 else x_bf[nxt]
            xTn = xT_bf[nxt]
            last = (l == NL - 1)
            with nc.named_scope(f"l{l}c{ts}_p4b"):
                for q in range(nQ):
                    t = nQ * ts + q
                    xr = spL.tile([P, D], F32, tag="lnr", name="xr", bufs=2)
                    nc.gpsimd.tensor_tensor(xr[:], x_f32[t][:],
                                            pin[:, q * D:(q + 1) * D],
                                            ALU.add)
                    nm_, rs = ln_stats_nm(xr[:])
                    if not last:
                        nc.gpsimd.tensor_scalar(x_f32[t][:], xr[:], nm_[:],
                                                rs[:], ALU.add, ALU.mult)
                    if YKV_FP8 and not last:
                        nc.vector.tensor_scalar(
                            xb2[nxt][t // 2][:, t % 2, :], xr[:],
                            nm_[:], rs[:], ALU.add, ALU.mult)
                    if (not YKV_FP8) and not last:
                        nc.gpsimd.tensor_scalar(xbn[t][:], xr[:], nm_[:],
                                                rs[:], ALU.add, ALU.mult)
                    # bf16 staging for the DMA transpose (XBAR needs 2-byte)
                    xstg = spC.tile([P, D], mm_dt, tag="xstg", name="xstg",
                                    bufs=5)
                    nc.gpsimd.tensor_scalar(xstg[:], xr[:], nm_[:], rs[:],
                                            ALU.add, ALU.mult)
                    for d in range(nD):
                        if pe_tp:
                            # tail path: XBAR transposes would serialize
                            # ~10us on the sync queue with nothing to hide
                            # them; PE+ACT are idle here instead
                            # ppb slot reuse: [P, 2*SUP] bf16 == [P, SUP]
                            # f32 in bytes, so it shares the "big" ring
                            xpt = ppb.tile([P, 2 * SUP], mm_dt, tag="big",
                                           name="xpt")
                            nc.tensor.transpose(
                                xpt[:, 0:P], xstg[:, d * P:(d + 1) * P],
                                ident_sb[:])
                            nc.scalar.activation(
                                xTn[d][:, t * P:(t + 1) * P], xpt[:, 0:P],
                                AF.Copy)
                        else:
                            _pend_tp.append(
                                (xTn[d][:, t * P:(t + 1) * P],
                                 xstg[:, d * P:(d + 1) * P]))
                        if P1_FP8 and not last:
                            _pend_cvt[0].append(
                                (xTn[d][:, t * P:(t + 1) * P],
                                 xTp[nxt][:, d, t * P:(t + 1) * P]))
            if last:
                fins.append(ts)

        def flush_tp():
            for dst, src in _pend_tp:
                nc.sync.dma_start(dst, src, transpose=True)
            _pend_tp.clear()

        def emit_final(ts):
            xTf = xT_bf[NL % 2]
            with nc.named_scope(f"fin{ts}"):
                och = spC.tile([P, nQ * V], F32, tag="och", name="och")
                o_ps = [ppw.tile([P, 2 * V], F32, tag="wide", name=f"o{h}")
                        for h in range(nQ // 2)]
                for q in range(nQ):
                    t = nQ * ts + q
                    dst = o_ps[q // 2][:, (q % 2) * V:(q % 2 + 1) * V]
                    for d in range(nD):
                        nc.tensor.matmul(dst,
                                         xTf[d][:, t * P:(t + 1) * P],
                                         wlm_sb[d][:],
                                         start=(d == 0 and q % 2 == 0),
                                         stop=(d == nD - 1 and q % 2 == 1))
                for q in range(nQ):
                    nc.scalar.activation(
                        och[:, q * V:(q + 1) * V],
                        o_ps[q // 2][:, (q % 2) * V:(q % 2 + 1) * V], AF.Copy)
                nc.scalar.dma_start(
                    out_d[ts * SUP:(ts + 1) * SUP, :].rearrange(
                        "(n p) v -> p n v", p=P),
                    och[:].rearrange("p (n v) -> p n v", n=nQ))

        # ---- flattened chunk pipeline ----
        n_chunks = NL * nTS

        if nTS == 1:
            setup_late()
            for i in range(n_chunks):
                if i >= 1:
                    emit_p4a(i - 1, 0)
                    emit_p4b()
                    flush_tp()
                    flush_cvt()
                    if fins:
                        emit_final(fins.pop(0))
                emit_p1(i, 0)
                emit_p2(i, 0)
                emit_p3(i, 0)
                emit_ar(i, 0)
            emit_p4a(NL - 1, 0)
            emit_p4b()
            flush_tp()
            flush_cvt()
            for pts in fins:
                emit_final(pts)
        else:
            # p2 leads each iteration (its qrT was roped last iteration);
            # p1 of the NEXT chunk fills the PE while p2's ykvT stats and
            # p3's operands settle.
            emit_p1(0, 0)
            setup_late()
            for i in range(n_chunks):
                l, ts = divmod(i, nTS)
                emit_p2(l, ts)
                nxt = divmod(i + 1, nTS) if i + 1 < n_chunks else None
                emit_p13(l, ts, nxt)
                emit_ar(l, ts)
                if i >= 1:
                    emit_p4a(*divmod(i - 1, nTS))
                if len(fins) >= 2:
                    emit_final(fins.pop(0))
            emit_p4a(NL - 1, nTS - 1)
            # tail: each ready fin goes out BEFORE the next (AR-gated) p4b
            # so lm_head matmuls overlap the last AllReduce waits
            while _pend_p4 or fins:
                if fins:
                    emit_final(fins.pop(0))
                if _pend_p4:
                    emit_p4b(pe_tp=True)
                    flush_tp()

        for cm in reversed(_cms):
            cm.__exit__(None, None, None)
        for f in reversed(_keep):
            f()
        _keep.clear()

    nc.compile()
    return nc


def host_inputs(idx, embed, encoder, encoder_v, decoder, lm_head, cfg,
                mm_dt=BF16):
    """Build the 8 per-core input maps (host-side prep is O(MB) copies)."""
    T, D, N, NL, V = cfg["T"], cfg["D"], cfg["N"], cfg["NL"], cfg["V"]
    NH, B = cfg["NH"], cfg["B"]
    np_mm = np.dtype(mybir.dt.np(mm_dt))
    np_f8 = np.dtype(mybir.dt.np(FP8))
    enc_np = np_f8 if P1_FP8 else np_mm
    ykv_np = np_f8 if YS_FP8 else np_mm

    idx = np.asarray(idx)
    embed = np.asarray(embed, dtype=np.float32)
    encoder = np.asarray(encoder, dtype=np.float32)
    encoder_v = np.asarray(encoder_v, dtype=np.float32)
    decoder = np.asarray(decoder, dtype=np.float32)
    lm_head = np.asarray(lm_head, dtype=np.float32)

    # initial x = ln(embed[idx]) in f32 (cheap: B*T*D)
    e = embed[idx]  # (B, T, D)
    mu = e.mean(-1, keepdims=True)
    var = ((e - mu) ** 2).mean(-1, keepdims=True)
    x0 = ((e - mu) / np.sqrt(var + EPS)).astype(np.float32)

    # rope tables in pair-permuted transposed layout [N/2, T]
    theta = np.float32(2.0 ** 16)
    q = (np.floor(np.arange(N, dtype=np.float32) / 2.0) * 2.0).astype(np.float32)
    freqs = (1.0 / (theta ** (q / np.float32(N))) /
             np.float32(2.0 * math.pi)).astype(np.float32)
    fp = freqs[0::2]  # (N/2,)
    ph = fp[:, None] * np.arange(T, dtype=np.float32)[None, :]
    pm = ((ph % np.float32(1.0)) * np.float32(2.0 * math.pi)).astype(np.float32)
    ctab = np.cos(pm).astype(np_mm)
    stab = np.sin(pm).astype(np_mm)

    perm = np.concatenate([np.arange(0, N, 2), np.arange(1, N, 2)])
    maskt = np.triu(np.ones((P, P), np.float32), k=1).astype(np_mm)  # s < t

    def dpair(a, dt):
        # (D, M) -> (128, 2*M) with d-pair interleave (d, d+128)
        m = a.shape[1]
        return np.stack([a[0:P, :], a[P:2 * P, :]], axis=1).reshape(
            P, 2 * m).astype(dt)

    in_maps = []
    for c in range(B * NH):
        b, h = c // NH, c % NH
        x0b = x0[b].astype(np_mm)
        x0t = np.ascontiguousarray(x0[b].T)
        in_maps.append({
            "x0": x0[b],
            "x0b": x0b,
            "x0b8": x0[b].astype(np_f8),
            "x0t": x0t.astype(np_mm),
            "x0tp": dpair(x0t, np_f8),
            "wencp": dpair(encoder[h][:, perm], enc_np),
            "wencv": dpair(encoder_v[h][:, perm], ykv_np),
            "wdec": decoder[h * N:(h + 1) * N, :][perm, :].astype(np_mm),
            "wlm": lm_head.astype(np_mm),
            "ctab": ctab,
            "stab": stab,
            "maskt": maskt,
            "ident": np.eye(P, dtype=np.float32).astype(np_mm),
        })
    return in_maps


_NC_CACHE = {}


def _get_nc(cfg_key, cfg, mm_dt, n_cores):
    if cfg_key not in _NC_CACHE:
        _NC_CACHE[cfg_key] = build_nc(cfg, mm_dt=mm_dt, n_cores=n_cores)
    return _NC_CACHE[cfg_key]


def kernel(idx, embed, encoder, encoder_v, decoder, lm_head):
    cfg = FULL_CFG
    NH, B = cfg["NH"], cfg["B"]
    n_cores = B * NH
    in_maps = host_inputs(idx, embed, encoder, encoder_v, decoder, lm_head, cfg)
    nc = _get_nc("full_bf16", cfg, BF16, n_cores)
    res = run_bass_kernel_spmd(nc, in_maps, core_ids=list(range(n_cores)))
    out = np.stack([np.asarray(res.results[b * NH]["out"], dtype=np.float32)
                    for b in range(B)], axis=0)
    return out


# revision 39
# speedup vs baseline: 1.2303x; 1.2303x over previous
"""Trainium2 Bass kernel for the sparse-attention nn module (nn_BDH_48421461295735).

Strategy: 8 NeuronCores = 8 (batch, head) pairs (B=2 x NH=4).  Each core runs
all 4 layers for its head; the only cross-core traffic is a per-layer
AllReduce (within each batch's group of 4 cores) of the per-head decoder
partial p = (x_sparse*y_sparse) @ dec_h, chunked by 512 token rows (bf16).

v4 over v2 (1.15 ms):
  - optional fp8e4+DoubleRow for the p1 (wenc @ xT), y_sparse
    (wencv @ ykvT) and ykv (x^T @ scoresT) matmuls, halving their PE
    cycles.  Flags P1_FP8 / YS_FP8 / YKV_FP8 pick the set (the lm_head
    and final-x stay bf16: quantizing them costs ~1.8e-2 rel err).
  - p1's fp8 xT is produced by converting the bf16 DMA-transpose output;
    the converts are DEFERRED one pipeline iteration (emitted at the top
    of the next p13) so p4's ACT ops never head-of-line block p2's
    score-staging copies (the v2/v3 traces showed ~10us PE stalls per
    chunk from exactly that).
  - p2 emits the ykvTp copies before the stats squares so p3's first
    y_sparse matmul is not queued behind them.
  - final lm_head chunks reordered so fin(ts) never waits on the last
    p4's transposes.
"""

import math
import sys

import numpy as np

for _p in ("/opt/trn_rl_repo",):
    if _p not in sys.path:
        sys.path.insert(0, _p)

import concourse.bass as bass
import concourse.bacc as bacc
import concourse.mybir as mybir
import concourse.tile as tile
from concourse.tile_rust import add_dep_helper
from concourse.bass_utils import run_bass_kernel_spmd

F32 = mybir.dt.float32
BF16 = mybir.dt.bfloat16
FP8 = mybir.dt.float8e4
AF = mybir.ActivationFunctionType
ALU = mybir.AluOpType
DR = mybir.MatmulPerfMode.DoubleRow

FULL_CFG = dict(T=2048, D=256, N=2048, NL=4, V=256, NH=4, B=2)
P = 128
SUP = 512
EPS = 1e-5
YKV_SC = 64.0  # ykvT pre-scale so fp8e4 never saturates (folded into rs4)

P1_FP8 = False   # x_sparse matmul via fp8 DoubleRow (wenc, xT in fp8)
YS_FP8 = False   # y_sparse matmul via fp8 DoubleRow (wencv, ykvT in fp8)
YKV_FP8 = True   # ykv matmul via fp8 DoubleRow (x, scoresT in fp8)


def build_nc(cfg, mm_dt=BF16, n_cores=8):
    T, D, N, NL, V = cfg["T"], cfg["D"], cfg["N"], cfg["NL"], cfg["V"]
    NH = cfg["NH"]
    assert T % SUP == 0 and D % P == 0 and N % 256 == 0 and V == D
    nTB, nTS, nD, nK = T // P, T // SUP, D // P, N // P
    nJ = nK // 2
    nQ = SUP // P  # 4
    assert nD == 2
    enc_dt = FP8 if P1_FP8 else mm_dt
    ykv_dt = FP8 if YS_FP8 else mm_dt
    st_dt = FP8 if YKV_FP8 else mm_dt

    nc = bacc.Bacc("TRN2", target_bir_lowering=False, debug=False,
                   num_devices=n_cores)

    x0_d = nc.dram_tensor("x0", [T, D], F32, kind="ExternalInput")
    x0b_d = nc.dram_tensor("x0b", [T, D], mm_dt, kind="ExternalInput")
    x0b8_d = nc.dram_tensor("x0b8", [T, D], FP8, kind="ExternalInput")
    x0t_d = nc.dram_tensor("x0t", [D, T], mm_dt, kind="ExternalInput")
    x0tp_d = nc.dram_tensor("x0tp", [P, 2 * T], FP8, kind="ExternalInput")
    wencp_d = nc.dram_tensor("wencp", [P, 2 * N], enc_dt, kind="ExternalInput")
    wencv_d = nc.dram_tensor("wencv", [P, 2 * N], ykv_dt, kind="ExternalInput")
    wdec_d = nc.dram_tensor("wdec", [N, D], mm_dt, kind="ExternalInput")
    wlm_d = nc.dram_tensor("wlm", [D, V], mm_dt, kind="ExternalInput")
    ctab_d = nc.dram_tensor("ctab", [N // 2, T], mm_dt, kind="ExternalInput")
    stab_d = nc.dram_tensor("stab", [N // 2, T], mm_dt, kind="ExternalInput")
    maskt_d = nc.dram_tensor("maskt", [P, P], mm_dt, kind="ExternalInput")
    ident_d = nc.dram_tensor("ident", [P, P], mm_dt, kind="ExternalInput")
    out_d = nc.dram_tensor("out", [T, V], F32, kind="ExternalOutput")

    # AllReduce groups: one group of NH cores per batch.
    RG = [list(range(g * NH, (g + 1) * NH)) for g in range(max(1, n_cores // NH))]

    with tile.TileContext(nc) as tc:
        _keep = []  # keep tc.tile free-closures alive (GC would release pools)

        def ptile(shape, dtype, name, **kw):
            t, free = tc.tile(shape, dtype, name=name, **kw)
            _keep.append(free)
            return t

        # ---- persistent SBUF tensors ----
        wencp_sb = ptile([P, 2, N], enc_dt, name="wencp")  # d-pair layout
        wencvp = ptile([P, 2, N], ykv_dt, name="wencvp")   # d-pair layout
        wdec_sb = [ptile([P, D], mm_dt, name=f"wdec{k}") for k in range(nK)]
        wlm_sb = [ptile([P, V], mm_dt, name=f"wlm{d}") for d in range(nD)]
        maskt_sb = ptile([P, P], mm_dt, name="maskt")
        x_f32 = [ptile([P, D], F32, name=f"xf{t}") for t in range(nTB)]
        # x_bf / xT_bf double-buffered by layer parity (p4 writes the other)
        if not YKV_FP8:
            x_bf = [[ptile([P, D], mm_dt, name=f"xb{pp}_{t}")
                     for t in range(nTB)] for pp in range(2)]
        xT_bf = [[ptile([P, T], mm_dt, name=f"xT{pp}_{d}") for d in range(nD)]
                 for pp in range(2)]
        if P1_FP8:
            # fp8 d-pair transposed x for the p1 DoubleRow matmuls
            xTp = [ptile([P, 2, T], FP8, name=f"xTp{pp}") for pp in range(2)]
        if YKV_FP8:
            # fp8 token-block-pair x for the ykv DoubleRow matmuls
            xb2 = [[ptile([P, 2, D], FP8, name=f"xb2_{pp}_{m}")
                    for m in range(nTB // 2)] for pp in range(2)]
        # qrT pair tiles: [:,0,:] = even half (pair group j), [:,1,:] = odd
        qp = [ptile([P, 2, T], FP8, name=f"qp{j}") for j in range(nJ)]
        # ykvT in d-pairs (scaled by 1/YKV_SC; the scale commutes out
        # through relu/decoder and is folded into rs4)
        ykvTp = ptile([P, 2, T], ykv_dt, name="ykvTp")
        # x_sparse stash, double-buffered by chunk parity
        xs_sb = [[ptile([P, SUP], mm_dt, name=f"xs{g}_{k}") for k in range(nK)]
                 for g in range(2)]
        eps_sb = ptile([P, 1], F32, name="epsb")
        nc.vector.memset(eps_sb[:], EPS)
        epsk_sb = ptile([P, 1], F32, name="epskb")
        nc.vector.memset(epsk_sb[:], EPS / (YKV_SC * YKV_SC))
        ones_sb = ptile([P, 1], mm_dt, name="onesb")
        nc.vector.memset(ones_sb[:], 1.0)
        ident_sb = ptile([P, P], mm_dt, name="identb")

        # per-layer DRAM bounce buffers for the chunked AllReduce (bf16)
        p_loc = [ptile([T, D], mm_dt, space="DRAM", name=f"ploc{l}")
                 for l in range(NL)]
        p_sum = [ptile([T, D], mm_dt, space="DRAM", addr_space="Shared",
                       name=f"psum{l}") for l in range(NL)]

        # transient pools
        _cms = [tc.tile_pool(name="spT", bufs=8),     # rope tables
                tc.tile_pool(name="spR", bufs=3),     # rope temporaries
                tc.tile_pool(name="spS", bufs=4),     # st staging, xy
                tc.tile_pool(name="spC", bufs=2),     # pch/pin/och staging
                tc.tile_pool(name="spL", bufs=8),     # LN scalars + scratch
                tc.tile_pool(name="ppb", bufs=4, space="PSUM"),
                tc.tile_pool(name="ppw", bufs=2, space="PSUM"),
                tc.tile_pool(name="pss", bufs=1, space="PSUM"),
                tc.tile_pool(name="ppr", bufs=1, space="PSUM")]
        spT, spR, spS, spC, spL, ppb, ppw, pss, ppr = \
            [cm.__enter__() for cm in _cms]

        def ln_stats_nm(src_ap):
            """LN stats over free dim D of [P, D] f32/psum AP.
            Returns (nm, rs): per-partition -mu and 1/sd."""
            s1 = spL.tile([P, 1], F32, tag="ln1", name="s1")
            nc.vector.reduce_sum(s1[:], src_ap, axis=mybir.AxisListType.X)
            nm = spL.tile([P, 1], F32, tag="ln2", name="nm")
            nc.vector.tensor_scalar_mul(nm[:], s1[:], -1.0 / D)
            sq = spL.tile([P, D], BF16, tag="lnsq", name="sq", bufs=2)
            ss = spL.tile([P, 1], F32, tag="ln3", name="ss")
            nc.scalar.activation(sq[:], src_ap, AF.Square, bias=nm[:],
                                 accum_out=ss[:])
            sd = spL.tile([P, 1], F32, tag="ln4", name="sd")
            nc.scalar.activation(sd[:], ss[:], AF.Sqrt, bias=eps_sb[:],
                                 scale=1.0 / D)
            rs = spL.tile([P, 1], F32, tag="ln5", name="rs")
            nc.vector.reciprocal(rs[:], sd[:])
            return nm, rs

        # ---- setup: pure DMAs; only what p1(0,0)/p2(0,0) needs goes on the
        # sync queue (which also carries the first rope tables) ----
        with nc.named_scope("setup"):
            nc.sync.dma_start(
                wencp_sb[:].rearrange("p a n -> p (a n)"), wencp_d[:, :])
            if P1_FP8:
                nc.sync.dma_start(
                    xTp[0][:].rearrange("p a t -> p (a t)"), x0tp_d[:, :])
            else:
                for d in range(nD):
                    nc.sync.dma_start(xT_bf[0][d][:],
                                      x0t_d[d * P:(d + 1) * P, :])
            if YKV_FP8:
                for m in range(nTB // 2):
                    for h in range(2):
                        t = 2 * m + h
                        nc.scalar.dma_start(xb2[0][m][:, h, :],
                                            x0b8_d[t * P:(t + 1) * P, :])
            else:
                for t in range(nTB):
                    nc.scalar.dma_start(x_bf[0][t][:],
                                        x0b_d[t * P:(t + 1) * P, :])
            nc.scalar.dma_start(maskt_sb[:], maskt_d[:, :])

        def setup_late():
            # needed only from p3(0,0) onwards: emitted after p1(0,0) so
            # chunk 0's relus aren't queued behind these DMA issues
            with nc.named_scope("setup2"):
                nc.scalar.dma_start(
                    wencvp[:].rearrange("p a n -> p (a n)"), wencv_d[:, :])
                for k in range(nK):
                    nc.scalar.dma_start(wdec_sb[k][:],
                                        wdec_d[k * P:(k + 1) * P, :])
                for t in range(nTB):
                    nc.scalar.dma_start(x_f32[t][:],
                                        x0_d[t * P:(t + 1) * P, :])
                for d in range(nD):
                    nc.scalar.dma_start(wlm_sb[d][:],
                                        wlm_d[d * P:(d + 1) * P, :])
                nc.scalar.dma_start(ident_sb[:], ident_d[:, :])

        def rope_j(j, l1, xs, c0, c1):
            """x_sparse matmuls for pair-group j + relu + rope (v2 scheme:
            DVE reads at most one SBUF tensor; gpsimd does SBUF*SBUF)."""
            par = l1 % 2
            ct = spT.tile([P, SUP], mm_dt, tag="ctc", name="ct")
            st = spT.tile([P, SUP], mm_dt, tag="stc", name="st")
            nc.sync.dma_start(ct[:], ctab_d[j * P:(j + 1) * P, c0:c1])
            nc.sync.dma_start(st[:], stab_d[j * P:(j + 1) * P, c0:c1])
            psA = ppb.tile([P, SUP], F32, tag="big", name="psA")
            psB = ppb.tile([P, SUP], F32, tag="big", name="psB")
            if P1_FP8:
                xT = xTp[par]
                nc.tensor.matmul(psA[:], wencp_sb[:, :, j * P:(j + 1) * P],
                                 xT[:, :, c0:c1], start=True, stop=True,
                                 perf_mode=DR)
                nc.tensor.matmul(
                    psB[:], wencp_sb[:, :, (j + nJ) * P:(j + nJ + 1) * P],
                    xT[:, :, c0:c1], start=True, stop=True, perf_mode=DR)
            else:
                xT = xT_bf[par]
                for d in range(nD):
                    nc.tensor.matmul(
                        psA[:], wencp_sb[:, d, j * P:(j + 1) * P],
                        xT[d][:, c0:c1], start=(d == 0), stop=(d == nD - 1))
                for d in range(nD):
                    nc.tensor.matmul(
                        psB[:], wencp_sb[:, d, (j + nJ) * P:(j + nJ + 1) * P],
                        xT[d][:, c0:c1], start=(d == 0), stop=(d == nD - 1))
            xsE, xsO = xs[j], xs[j + nJ]
            nc.scalar.activation(xsE[:], psA[:], AF.Relu)
            nc.scalar.activation(xsO[:], psB[:], AF.Relu)
            # rope: qE = relu(A)*c - relu(B)*s ; qO = relu(B)*c + relu(A)*s
            t0 = ppr.tile([P, SUP], F32, tag="tr", name="t0")
            t1 = spR.tile([P, SUP], mm_dt, tag="t1", name="t1")
            t2 = spR.tile([P, SUP], mm_dt, tag="t2", name="t2")
            qE, qO = qp[j][:, 0, c0:c1], qp[j][:, 1, c0:c1]
            nc.vector.scalar_tensor_tensor(t0[:], psA[:], 0.0, ct[:],
                                           ALU.max, ALU.mult)
            nc.gpsimd.tensor_tensor(t1[:], xsO[:], st[:], ALU.mult)
            nc.vector.tensor_tensor(qE, t0[:], t1[:], ALU.subtract)
            nc.gpsimd.tensor_tensor(t2[:], xsO[:], ct[:], ALU.mult)
            t3 = ppr.tile([P, SUP], F32, tag="tr", name="t3")
            nc.vector.scalar_tensor_tensor(t3[:], psA[:], 0.0, st[:],
                                           ALU.max, ALU.mult)
            nc.vector.tensor_tensor(qO, t3[:], t2[:], ALU.add)

        def emit_p1(l, ts):
            xs = xs_sb[ts % 2]
            c0, c1 = ts * SUP, (ts + 1) * SUP
            with nc.named_scope(f"l{l}c{ts}_p1"):
                for j in range(nJ):
                    rope_j(j, l, xs, c0, c1)

        _rs4 = [None]

        def emit_p2(l, ts):
            par = l % 2
            c0, c1 = ts * SUP, (ts + 1) * SUP
            with nc.named_scope(f"l{l}c{ts}_p2"):
                # ykv is computed directly in transposed layout:
                #   ykvT[d, t] = sum_s x[s, d] * scoresT[s, t]
                # The per-token LN of ykv needs no mean (ykv is exactly
                # zero-mean since x is layer-normed), and the 1/sd scale
                # commutes through relu/encv/decoder, so it is applied to
                # p's rows (per-partition scale) in p3's pch copy instead.
                yT_ps = [ppw.tile([P, SUP], F32, tag="wide", name=f"yT{d}")
                         for d in range(nD)]
                nsb = nQ * ts + nQ

                def score_mms(sb):
                    r = sb - nQ * ts
                    q0 = max(0, r)
                    st_ps = ppb.tile([P, SUP], F32, tag="big", name="st_ps")
                    dst = st_ps[:, q0 * P:SUP]
                    for j in range(nJ):
                        nc.tensor.matmul(
                            dst, qp[j][:, :, sb * P:(sb + 1) * P],
                            qp[j][:, :, c0 + q0 * P:c1],
                            start=(j == 0), stop=(j == nJ - 1),
                            perf_mode=DR)
                    return st_ps, r

                def stage_copy(dsl, st_ps, r):
                    # masked copy of one score block into SBUF staging;
                    # dsl(a, b) yields the [P, b-a] destination column slice
                    if r >= 0:
                        if r > 0:
                            nc.gpsimd.memset(dsl(0, r * P), 0.0)
                        nc.vector.tensor_tensor(
                            dsl(r * P, (r + 1) * P),
                            st_ps[:, r * P:(r + 1) * P], maskt_sb[:],
                            ALU.mult)
                        if r + 1 < nQ:
                            nc.scalar.activation(dsl((r + 1) * P, SUP),
                                                 st_ps[:, (r + 1) * P:SUP],
                                                 AF.Copy)
                    else:
                        nc.scalar.activation(dsl(0, SUP), st_ps[:], AF.Copy)

                if YKV_FP8:
                    xb = xb2[par]
                    prev = None
                    for pbi in range(nsb // 2):
                        stp = spS.tile([P, 2, SUP], FP8, tag="stp",
                                       name="stp")
                        for h in range(2):
                            st_ps, r = score_mms(2 * pbi + h)
                            stage_copy(lambda a, b, hh=h: stp[:, hh, a:b],
                                       st_ps, r)
                        # ykv lags one score pair so the PE never waits on
                        # the staging copies it just requested
                        if prev is not None:
                            ppi, pstp = prev
                            for d in range(nD):
                                nc.tensor.matmul(
                                    yT_ps[d][:],
                                    xb[ppi][:, :, d * P:(d + 1) * P],
                                    pstp[:, :, :],
                                    start=(ppi == 0), stop=False,
                                    perf_mode=DR)
                        prev = (pbi, stp)
                    ppi, pstp = prev
                    for d in range(nD):
                        nc.tensor.matmul(
                            yT_ps[d][:], xb[ppi][:, :, d * P:(d + 1) * P],
                            pstp[:, :, :],
                            start=(ppi == 0), stop=True, perf_mode=DR)
                else:
                    xb = x_bf[par]
                    prev = None
                    for si in range(nsb):
                        st_ps, r = score_mms(si)
                        st_sb = spS.tile([P, SUP], mm_dt, tag="stp",
                                         name="st_sb")
                        stage_copy(lambda a, b: st_sb[:, a:b], st_ps, r)
                        if prev is not None:
                            psi, pst = prev
                            for d in range(nD):
                                nc.tensor.matmul(
                                    yT_ps[d][:],
                                    xb[psi][:, d * P:(d + 1) * P],
                                    pst[:], start=(psi == 0), stop=False)
                        prev = (si, st_sb)
                    psi, pst = prev
                    for d in range(nD):
                        nc.tensor.matmul(
                            yT_ps[d][:], xb[psi][:, d * P:(d + 1) * P],
                            pst[:], start=(psi == 0), stop=True)

                # ykvTp copies first: p3's first y_sparse matmul needs them
                for d in range(nD):
                    nc.scalar.activation(ykvTp[:, d, c0:c1], yT_ps[d][:],
                                         AF.Copy, scale=1.0 / YKV_SC)
                # stats: ss4[:, q] = sum_d ykvT^2 via ones-matmul, then
                # rs4 = 1/sqrt(ss/D + eps) stays in token-column layout.
                sqs = []
                for d in range(nD):
                    sq = spS.tile([P, SUP], mm_dt, tag=f"sq{d}", name="sq",
                                  bufs=2)
                    nc.scalar.activation(sq[:], yT_ps[d][:], AF.Square)
                    sqs.append(sq)
                ss4 = pss.tile([P, nQ], F32, tag="ss", name="ss4")
                for q in range(nQ):
                    for d in range(nD):
                        nc.tensor.matmul(
                            ss4[:, q:q + 1], sqs[d][:, q * P:(q + 1) * P],
                            ones_sb[:],
                            start=(q == 0 and d == 0),
                            stop=(q == nQ - 1 and d == nD - 1))
                sd4 = spL.tile([P, nQ], F32, tag="sd4", name="sd4")
                nc.scalar.activation(sd4[:], ss4[:], AF.Sqrt, bias=epsk_sb[:],
                                     scale=1.0 / (D * YKV_SC * YKV_SC))
                rs4 = spL.tile([P, nQ], F32, tag="rs4", name="rs4")
                nc.vector.reciprocal(rs4[:], sd4[:])
                _rs4[0] = rs4

        _last_pch = [None]

        def p3_step(k, xs3, c30, c31, p_ps):
            ys_ps = ppb.tile([P, SUP], F32, tag="big", name="ys_ps")
            if YS_FP8:
                nc.tensor.matmul(ys_ps[:], wencvp[:, :, k * P:(k + 1) * P],
                                 ykvTp[:, :, c30:c31], start=True, stop=True,
                                 perf_mode=DR)
            else:
                for d in range(nD):
                    nc.tensor.matmul(ys_ps[:],
                                     wencvp[:, d, k * P:(k + 1) * P],
                                     ykvTp[:, d, c30:c31],
                                     start=(d == 0), stop=(d == nD - 1))
            xy = spS.tile([P, SUP], mm_dt, tag="xy", name="xy")
            # fused relu+mult on DVE (gpsimd cannot read PSUM)
            nc.vector.scalar_tensor_tensor(
                xy[:], ys_ps[:], 0.0, xs3[k][:], ALU.max, ALU.mult)
            for q in range(nQ):
                nc.tensor.matmul(
                    p_ps[q // 2][:, (q % 2) * D:(q % 2 + 1) * D],
                    xy[:, q * P:(q + 1) * P],
                    wdec_sb[k][:],
                    start=(k == 0 and q % 2 == 0),
                    stop=(k == nK - 1 and q % 2 == 1))

        def p3_tail(p_ps, rs4):
            pch = spC.tile([P, nQ * D], mm_dt, tag="pch", name="pch")
            for q in range(nQ):
                nc.scalar.activation(
                    pch[:, q * D:(q + 1) * D],
                    p_ps[q // 2][:, (q % 2) * D:(q % 2 + 1) * D],
                    AF.Copy, scale=rs4[:, q:q + 1])
            _last_pch[0] = pch

        def emit_p3(l, ts):
            xs3 = xs_sb[ts % 2]
            c30, c31 = ts * SUP, (ts + 1) * SUP
            with nc.named_scope(f"l{l}c{ts}_p3"):
                p_ps = [ppw.tile([P, SUP], F32, tag="wide", name=f"pp{h}")
                        for h in range(nQ // 2)]
                for k in range(nK):
                    p3_step(k, xs3, c30, c31, p_ps)
                p3_tail(p_ps, _rs4[0])

        # deferred bf16->fp8 xT converts: (src_slice, dst_slice) pairs
        _pend_cvt = [[]]

        def flush_cvt():
            for src, dst in _pend_cvt[0]:
                nc.scalar.activation(dst, src, AF.Copy)
            _pend_cvt[0] = []

        def emit_p13(l3, ts3, p1_next):
            """p3 of chunk (l3,ts3) interleaved with p1 of the next chunk."""
            xs3 = xs_sb[ts3 % 2]
            c30, c31 = ts3 * SUP, (ts3 + 1) * SUP
            rs4 = _rs4[0]
            if p1_next is not None:
                l1, ts1 = p1_next
                xs1 = xs_sb[ts1 % 2]
                c10, c11 = ts1 * SUP, (ts1 + 1) * SUP
            with nc.named_scope(f"l{l3}c{ts3}_p31"):
                flush_cvt()
                emit_p4b()
                p_ps = [ppw.tile([P, SUP], F32, tag="wide", name=f"pp{h}")
                        for h in range(nQ // 2)]
                for j in range(nJ):
                    if p1_next is not None:
                        rope_j(j, l1, xs1, c10, c11)
                    p3_step(2 * j, xs3, c30, c31, p_ps)
                    p3_step(2 * j + 1, xs3, c30, c31, p_ps)
                p3_tail(p_ps, rs4)
                flush_tp()

        _pch_dma = [None]

        def emit_ar(l, ts):
            c0, c1 = ts * SUP, (ts + 1) * SUP
            with nc.named_scope(f"l{l}c{ts}_ar"):
                # pch staged by p3 just above (same iteration).  High
                # priority: the collective's issue instruction must be
                # popped by the gpsimd queue the moment pch lands, else
                # the AR start (and its ~14us wire time) slides ~40us to
                # after the next chunk's rope multiplies.
                with tc.high_priority():
                    pd = nc.sync.dma_start(
                        p_loc[l][c0:c1, :].rearrange("(n p) d -> p n d", p=P),
                        _last_pch[0].rearrange("p (n d) -> p n d", n=nQ))
                    _pch_dma[0] = getattr(pd, "ins", None)
                    nc.gpsimd.collective_compute(
                        "AllReduce", ALU.add, replica_groups=RG,
                        ins=[p_loc[l][c0:c1, :]], outs=[p_sum[l][c0:c1, :]])

        # p4 is split: p4a issues only the pin DMA (one iteration after the
        # AR); p4b runs the LN chain + x writes at the TOP of the p13 two
        # iterations after the AR (when its completion is certain, so the
        # scheduler cannot head-of-line block any queue on it), and the
        # transposes at that p13's BOTTOM (their consumers are one more
        # iteration out).
        _pend_p4 = []   # (l, ts, pin_tile)
        _pend_tp = []   # (dst_slice, src_slice) DMA transposes
        fins = []       # last-layer chunks whose lm_head is pending

        def emit_p4a(l, ts):
            c0, c1 = ts * SUP, (ts + 1) * SUP
            with nc.named_scope(f"l{l}c{ts}_p4a"):
                pin = spC.tile([P, nQ * D], mm_dt, tag="pin", name="pin")
                pi = nc.sync.dma_start(
                    pin[:].rearrange("p (n d) -> p n d", n=nQ),
                    p_sum[l][c0:c1, :].rearrange("(n p) d -> p n d", p=P))
                # Pin the pin-DMA's *simulated* readiness to ~this
                # iteration's end (the current chunk's pch DMA): the cost
                # model underestimates when the AllReduce lands, and the
                # scheduler would otherwise place the whole AR-gated p4b
                # chain a full iteration early, where it head-of-line
                # blocks the DVE/gpsimd queues for ~10us at every layer
                # boundary.  Runtime cost is nil: its consumers have a
                # full iteration of slack.
                pii = getattr(pi, "ins", None)
                if pii is not None and _pch_dma[0] is not None and nTS > 1:
                    add_dep_helper(pii, _pch_dma[0], sync=False,
                                   reason="pin after current pch (placement)")
                _pend_p4.append((l, ts, pin))

        def emit_p4b(pe_tp=False):
            if not _pend_p4:
                return
            l, ts, pin = _pend_p4.pop(0)
            nxt = (l + 1) % 2
            xbn = None if YKV_FP8 else x_bf[nxt]
            xTn = xT_bf[nxt]
            last = (l == NL - 1)
            with nc.named_scope(f"l{l}c{ts}_p4b"):
                for q in range(nQ):
                    t = nQ * ts + q
                    xr = spL.tile([P, D], F32, tag="lnr", name="xr", bufs=2)
                    nc.gpsimd.tensor_tensor(xr[:], x_f32[t][:],
                                            pin[:, q * D:(q + 1) * D],
                                            ALU.add)
                    nm_, rs = ln_stats_nm(xr[:])
                    if not last:
                        nc.gpsimd.tensor_scalar(x_f32[t][:], xr[:], nm_[:],
                                                rs[:], ALU.add, ALU.mult)
                    if YKV_FP8 and not last:
                        nc.vector.tensor_scalar(
                            xb2[nxt][t // 2][:, t % 2, :], xr[:],
                            nm_[:], rs[:], ALU.add, ALU.mult)
                    if (not YKV_FP8) and not last:
                        nc.gpsimd.tensor_scalar(xbn[t][:], xr[:], nm_[:],
                                                rs[:], ALU.add, ALU.mult)
                    # bf16 staging for the DMA transpose (XBAR needs 2-byte)
                    xstg = spC.tile([P, D], mm_dt, tag="xstg", name="xstg",
                                    bufs=5)
                    nc.gpsimd.tensor_scalar(xstg[:], xr[:], nm_[:], rs[:],
                                            ALU.add, ALU.mult)
                    for d in range(nD):
                        if pe_tp:
                            # tail path: XBAR transposes would serialize
                            # ~10us on the sync queue with nothing to hide
                            # them; PE+ACT are idle here instead
                            # ppb slot reuse: [P, 2*SUP] bf16 == [P, SUP]
                            # f32 in bytes, so it shares the "big" ring
                            xpt = ppb.tile([P, 2 * SUP], mm_dt, tag="big",
                                           name="xpt")
                            nc.tensor.transpose(
                                xpt[:, 0:P], xstg[:, d * P:(d + 1) * P],
                                ident_sb[:])
                            nc.scalar.activation(
                                xTn[d][:, t * P:(t + 1) * P], xpt[:, 0:P],
                                AF.Copy)
                        else:
                            _pend_tp.append(
                                (xTn[d][:, t * P:(t + 1) * P],
                                 xstg[:, d * P:(d + 1) * P]))
                        if P1_FP8 and not last:
                            _pend_cvt[0].append(
                                (xTn[d][:, t * P:(t + 1) * P],
                                 xTp[nxt][:, d, t * P:(t + 1) * P]))
            if last:
                fins.append(ts)

        def flush_tp():
            for dst, src in _pend_tp:
                nc.sync.dma_start(dst, src, transpose=True)
            _pend_tp.clear()

        def emit_final(ts):
            xTf = xT_bf[NL % 2]
            with nc.named_scope(f"fin{ts}"):
                och = spC.tile([P, nQ * V], F32, tag="och", name="och")
                o_ps = [ppw.tile([P, 2 * V], F32, tag="wide", name=f"o{h}")
                        for h in range(nQ // 2)]
                for q in range(nQ):
                    t = nQ * ts + q
                    dst = o_ps[q // 2][:, (q % 2) * V:(q % 2 + 1) * V]
                    for d in range(nD):
                        nc.tensor.matmul(dst,
                                         xTf[d][:, t * P:(t + 1) * P],
                                         wlm_sb[d][:],
                                         start=(d == 0 and q % 2 == 0),
                                         stop=(d == nD - 1 and q % 2 == 1))
                for q in range(nQ):
                    nc.scalar.activation(
                        och[:, q * V:(q + 1) * V],
                        o_ps[q // 2][:, (q % 2) * V:(q % 2 + 1) * V], AF.Copy)
                nc.sync.dma_start(
                    out_d[ts * SUP:(ts + 1) * SUP, :].rearrange(
                        "(n p) v -> p n v", p=P),
                    och[:].rearrange("p (n v) -> p n v", n=nQ))

        # ---- flattened chunk pipeline ----
        n_chunks = NL * nTS

        if nTS == 1:
            setup_late()
            for i in range(n_chunks):
                if i >= 1:
                    emit_p4a(i - 1, 0)
                    emit_p4b()
                    flush_tp()
                    flush_cvt()
                    if fins:
                        emit_final(fins.pop(0))
                emit_p1(i, 0)
                emit_p2(i, 0)
                emit_p3(i, 0)
                emit_ar(i, 0)
            emit_p4a(NL - 1, 0)
            emit_p4b()
            flush_tp()
            flush_cvt()
            for pts in fins:
                emit_final(pts)
        else:
            # p2 leads each iteration (its qrT was roped last iteration);
            # p1 of the NEXT chunk fills the PE while p2's ykvT stats and
            # p3's operands settle.
            emit_p1(0, 0)
            setup_late()
            for i in range(n_chunks):
                l, ts = divmod(i, nTS)
                emit_p2(l, ts)
                nxt = divmod(i + 1, nTS) if i + 1 < n_chunks else None
                emit_p13(l, ts, nxt)
                emit_ar(l, ts)
                if i >= 1:
                    emit_p4a(*divmod(i - 1, nTS))
                if len(fins) >= 2:
                    emit_final(fins.pop(0))
            emit_p4a(NL - 1, nTS - 1)
            # tail: each ready fin goes out BEFORE the next (AR-gated) p4b
            # so lm_head matmuls overlap the last AllReduce waits
            while _pend_p4 or fins:
                if fins:
                    emit_final(fins.pop(0))
                if _pend_p4:
                    emit_p4b(pe_tp=True)
                    flush_tp()

        for cm in reversed(_cms):
            cm.__exit__(None, None, None)
        for f in reversed(_keep):
            f()
        _keep.clear()

    nc.compile()
    return nc


def host_inputs(idx, embed, encoder, encoder_v, decoder, lm_head, cfg,
                mm_dt=BF16):
    """Build the 8 per-core input maps (host-side prep is O(MB) copies)."""
    T, D, N, NL, V = cfg["T"], cfg["D"], cfg["N"], cfg["NL"], cfg["V"]
    NH, B = cfg["NH"], cfg["B"]
    np_mm = np.dtype(mybir.dt.np(mm_dt))
    np_f8 = np.dtype(mybir.dt.np(FP8))
    enc_np = np_f8 if P1_FP8 else np_mm
    ykv_np = np_f8 if YS_FP8 else np_mm

    idx = np.asarray(idx)
    embed = np.asarray(embed, dtype=np.float32)
    encoder = np.asarray(encoder, dtype=np.float32)
    encoder_v = np.asarray(encoder_v, dtype=np.float32)
    decoder = np.asarray(decoder, dtype=np.float32)
    lm_head = np.asarray(lm_head, dtype=np.float32)

    # initial x = ln(embed[idx]) in f32 (cheap: B*T*D)
    e = embed[idx]  # (B, T, D)
    mu = e.mean(-1, keepdims=True)
    var = ((e - mu) ** 2).mean(-1, keepdims=True)
    x0 = ((e - mu) / np.sqrt(var + EPS)).astype(np.float32)

    # rope tables in pair-permuted transposed layout [N/2, T]
    theta = np.float32(2.0 ** 16)
    q = (np.floor(np.arange(N, dtype=np.float32) / 2.0) * 2.0).astype(np.float32)
    freqs = (1.0 / (theta ** (q / np.float32(N))) /
             np.float32(2.0 * math.pi)).astype(np.float32)
    fp = freqs[0::2]  # (N/2,)
    ph = fp[:, None] * np.arange(T, dtype=np.float32)[None, :]
    pm = ((ph % np.float32(1.0)) * np.float32(2.0 * math.pi)).astype(np.float32)
    ctab = np.cos(pm).astype(np_mm)
    stab = np.sin(pm).astype(np_mm)

    perm = np.concatenate([np.arange(0, N, 2), np.arange(1, N, 2)])
    maskt = np.triu(np.ones((P, P), np.float32), k=1).astype(np_mm)  # s < t

    def dpair(a, dt):
        # (D, M) -> (128, 2*M) with d-pair interleave (d, d+128)
        m = a.shape[1]
        return np.stack([a[0:P, :], a[P:2 * P, :]], axis=1).reshape(
            P, 2 * m).astype(dt)

    in_maps = []
    for c in range(B * NH):
        b, h = c // NH, c % NH
        x0b = x0[b].astype(np_mm)
        x0t = np.ascontiguousarray(x0[b].T)
        in_maps.append({
            "x0": x0[b],
            "x0b": x0b,
            "x0b8": x0[b].astype(np_f8),
            "x0t": x0t.astype(np_mm),
            "x0tp": dpair(x0t, np_f8),
            "wencp": dpair(encoder[h][:, perm], enc_np),
            "wencv": dpair(encoder_v[h][:, perm], ykv_np),
            "wdec": decoder[h * N:(h + 1) * N, :][perm, :].astype(np_mm),
            "wlm": lm_head.astype(np_mm),
            "ctab": ctab,
            "stab": stab,
            "maskt": maskt,
            "ident": np.eye(P, dtype=np.float32).astype(np_mm),
        })
    return in_maps


_NC_CACHE = {}


def _get_nc(cfg_key, cfg, mm_dt, n_cores):
    if cfg_key not in _NC_CACHE:
        _NC_CACHE[cfg_key] = build_nc(cfg, mm_dt=mm_dt, n_cores=n_cores)
    return _NC_CACHE[cfg_key]


def kernel(idx, embed, encoder, encoder_v, decoder, lm_head):
    cfg = FULL_CFG
    NH, B = cfg["NH"], cfg["B"]
    n_cores = B * NH
    in_maps = host_inputs(idx, embed, encoder, encoder_v, decoder, lm_head, cfg)
    nc = _get_nc("full_bf16", cfg, BF16, n_cores)
    res = run_bass_kernel_spmd(nc, in_maps, core_ids=list(range(n_cores)))
    out = np.stack([np.asarray(res.results[b * NH]["out"], dtype=np.float32)
                    for b in range(B)], axis=0)
    return out
